# revision 10
# baseline (speedup 1.0000x reference)
"""Trainium2 Bass kernel for nn_ChenDifferentiableAllocator (entropic OT / Sinkhorn).

Reference computes, from trH[64], wmax[64], a[64], theta[64,6], phi[6], bits[6]:
    C    = 0.5*trH[:,None] * ((2*wmax[:,None]/(2^bits-1))^2 / 12)
    K    = -(C - theta)/0.02 ; b = softmax(phi)
    200x log-domain Sinkhorn(K, log a, log b); P = exp(K+f+g); P /= P.sum()

The Sinkhorn fixed point is reached (to well below fp32 resolution) in ~16
alternating updates (measured: rel change < 1e-11 by iteration 25 in f64), so
the kernel runs the mathematically-identical multiplicative form to
convergence instead of 200 log-domain steps:

    M  = exp(K); Ma = a*M (lhsT for column update); MbT = (M*b)^T (lhsT for
    row update); iterate  s = 1/(Mb t), t = 1/(Ma^T s);  then
    P = diag(s) (a*M*b) diag(t) / sum.

Global normalization makes every positive rescaling of a and b cancel, so b
is used unnormalized (exp(phi), no softmax denominator needed).

Per-core work is a strictly serial PE<->DVE ping-pong (matvec / reciprocal).
The problem is replicated on all 8 cores; core 0's output is returned.

All arithmetic happens on-device. Host only restages layouts: packs the
small vectors into two staging tensors, transposes theta, supplies a ones
vector. 2^bits-1 is computed exactly on-device with integer ops
((bits+127)<<23 bit-cast to fp32).
"""

import numpy as np

import concourse.bass as bass
import concourse.tile as tile
from concourse import bacc, mybir
from concourse.bass_utils import run_bass_kernel_spmd

F32 = mybir.dt.float32
I32 = mybir.dt.int32

L, B = 64, 6
EPS = 0.02
N_ITERS = 17
N_CORES = 8
N_WARM = 28  # dummy PE matmuls to trip HAM to 2.4 GHz before the loop

# pk_row layout (single partition strip): [trH(64) | wmax(64) | ones(64) | phi(6)]
_ROW_TRH = 0
_ROW_WMX = 64
_ROW_ONE = 128
_ROW_PHI = 192
_ROW_LEN = 198
# pk_col layout ([64, 3]): col0 = a, col1 = ones, col2 = phi padded to 64
_COL_LEN = 3


def _build():
    nc = bacc.Bacc("TRN2", target_bir_lowering=False, debug=False)

    pk_col_d = nc.dram_tensor("pk_col", [L, _COL_LEN], F32, kind="ExternalInput").ap()
    pk_row_d = nc.dram_tensor("pk_row", [1, _ROW_LEN], F32, kind="ExternalInput").ap()
    bits_d = nc.dram_tensor("bits", [B], I32, kind="ExternalInput").ap()
    theta_d = nc.dram_tensor("theta", [L, B], F32, kind="ExternalInput").ap()
    thetaT_d = nc.dram_tensor("thetaT", [B, L], F32, kind="ExternalInput").ap()
    out_d = nc.dram_tensor("out", [L, B], F32, kind="ExternalOutput").ap()

    with tile.TileContext(nc) as tc:
        _emit(tc, out_d, pk_col_d, pk_row_d, bits_d, theta_d, thetaT_d)

    nc.compile()
    return nc


def _emit(tc, out_d, pk_col_d, pk_row_d, bits_d, theta_d, thetaT_d):
    from contextlib import ExitStack

    nc = tc.nc
    ctx = ExitStack()
    with ctx:
        sg = ctx.enter_context(tc.tile_pool(name="sg", bufs=1))
        sp = ctx.enter_context(tc.tile_pool(name="sp", bufs=2))
        pp = ctx.enter_context(tc.tile_pool(name="pp", bufs=1, space="PSUM"))
        pr = ctx.enter_context(tc.tile_pool(name="pr", bufs=2, space="PSUM"))

        # ---- PE warm-up ----------------------------------------------------
        # The PE clock sits at 1.2 GHz (HAM-throttled) until ~3.4us of
        # sustained activity; the loop's duty cycle alone never trips it.
        # Run dep-free dummy matmuls during the input-DMA window so every
        # real matmul executes at 2.4 GHz.
        wsrc = sg.tile([1, 1], F32, tag="wsrc")
        nc.vector.memset(wsrc[:], 1.0)
        wps = pp.tile([1, 1], F32, tag="wm")
        for _ in range(N_WARM):
            nc.tensor.matmul(wps[:], wsrc[:], wsrc[:])

        # ---- input staging (spread across engine DMA queues) ---------------
        bits_r = sg.tile([1, B], I32, tag="bits_r")
        nc.sync.dma_start(bits_r[:], bits_d.unsqueeze(0))
        theta = sg.tile([L, B], F32, tag="theta")
        nc.sync.dma_start(theta[:], theta_d)
        pk_col = sg.tile([L, _COL_LEN], F32, tag="pk_col")
        nc.scalar.dma_start(pk_col[:], pk_col_d)
        pk_row = sg.tile([1, _ROW_LEN], F32, tag="pk_row")
        nc.scalar.dma_start(pk_row[:], pk_row_d)
        thetaT = sg.tile([B, L], F32, tag="thetaT")
        nc.gpsimd.dma_start(thetaT[:], thetaT_d)

        a_col = pk_col[:, 0:1]
        ones_col = pk_col[:, 1:2]
        phi_col = pk_col[0:B, 2:3]
        trh_row = pk_row[:, _ROW_TRH : _ROW_TRH + L]
        wmx_row = pk_row[:, _ROW_WMX : _ROW_WMX + L]
        ones_row = pk_row[:, _ROW_ONE : _ROW_ONE + L]
        phi_row = pk_row[:, _ROW_PHI : _ROW_PHI + B]

        # ---- preprocessing -------------------------------------------------
        # bcol/brow early: the first Exp triggers the ACT table load; let it
        # overlap the remaining input DMAs.
        bcol = sg.tile([B, 1], F32, tag="bcol")  # exp(phi), unnormalized b
        nc.scalar.activation(bcol[:], phi_col, mybir.ActivationFunctionType.Exp)
        brow = sg.tile([1, B], F32, tag="brow")
        nc.scalar.activation(brow[:], phi_row, mybir.ActivationFunctionType.Exp)

        # denom = 2^bits - 1, exactly: float_bits(2^b) = (b+127)*2^23, which is
        # exactly representable in fp32 (<= 8 mantissa bits), so compute it in
        # float, convert to int32, and bitcast.
        bits_f = sg.tile([1, B], F32, tag="bits_f")
        nc.vector.tensor_copy(bits_f[:], bits_r[:])
        ebits = sg.tile([1, B], F32, tag="ebits")
        nc.vector.tensor_scalar(
            ebits[:], bits_f[:], 127.0, float(1 << 23),
            mybir.AluOpType.add, mybir.AluOpType.mult,
        )
        p2i = sg.tile([1, B], I32, tag="p2i")
        nc.vector.tensor_copy(p2i[:], ebits[:])
        den = sg.tile([1, B], F32, tag="den")
        nc.vector.tensor_scalar(
            den[:], p2i[:].bitcast(F32), -1.0, None, mybir.AluOpType.add
        )
        densq = sg.tile([1, B], F32, tag="densq")
        nc.vector.tensor_tensor(densq[:], den[:], den[:], mybir.AluOpType.mult)
        colinv = sg.tile([1, B], F32, tag="colinv")  # 1/(2^b-1)^2
        nc.vector.reciprocal(colinv[:], densq[:])

        # rowe = trH*wmax^2/(6*EPS)  (row layout, for outer products)
        w2 = sg.tile([1, L], F32, tag="w2")
        nc.vector.tensor_tensor(w2[:], wmx_row, wmx_row, mybir.AluOpType.mult)
        rowe = sg.tile([1, L], F32, tag="rowe")
        nc.vector.scalar_tensor_tensor(
            rowe[:], w2[:], 1.0 / (6.0 * EPS), trh_row,
            mybir.AluOpType.mult, mybir.AluOpType.mult,
        )

        # Z1 = outer(rowe, colinv) [64,6];  Z2 = outer(colinv, rowe) [6,64]
        z1 = pp.tile([L, B], F32, tag="pa")
        nc.tensor.matmul(z1[:], rowe[:], colinv[:])
        z2 = pp.tile([B, L], F32, tag="pb")
        nc.tensor.matmul(z2[:], colinv[:], rowe[:])

        # K = theta/EPS - Z ;  M = exp(K) ;  MbT = exp(KT + phi_j) = (M*b)^T
        karg = sg.tile([L, B], F32, tag="karg")
        nc.vector.scalar_tensor_tensor(
            karg[:], theta[:], 1.0 / EPS, z1[:],
            mybir.AluOpType.mult, mybir.AluOpType.subtract,
        )
        kargT = sg.tile([B, L], F32, tag="kargT")
        nc.vector.scalar_tensor_tensor(
            kargT[:], thetaT[:], 1.0 / EPS, z2[:],
            mybir.AluOpType.mult, mybir.AluOpType.subtract,
        )
        m_mat = sg.tile([L, B], F32, tag="m_mat")
        nc.scalar.activation(m_mat[:], karg[:], mybir.ActivationFunctionType.Exp)
        mbT = sg.tile([B, L], F32, tag="mbT")  # lhsT of the row update
        nc.scalar.activation(
            mbT[:], kargT[:], mybir.ActivationFunctionType.Exp, bias=phi_col
        )
        ma = sg.tile([L, B], F32, tag="ma")  # lhsT of the column update
        nc.vector.tensor_scalar(
            ma[:], m_mat[:], a_col, None, mybir.AluOpType.mult
        )

        # ---- Sinkhorn loop -------------------------------------------------
        t_cur = sg.tile([B, 1], F32, tag="t0")
        nc.vector.reciprocal(t_cur[:], bcol[:])  # v0 = 1  =>  t0 = 1/b

        s_cur = None
        for i in range(N_ITERS):
            rp = pr.tile([L, 1], F32, tag="rp")
            nc.tensor.matmul(rp[:], mbT[:], t_cur[:])
            s_cur = sp.tile([L, 1], F32, tag="s")
            nc.vector.reciprocal(s_cur[:], rp[:])
            if i < N_ITERS - 1:
                cp = pr.tile([B, 1], F32, tag="cp")
                nc.tensor.matmul(cp[:], ma[:], s_cur[:])
                t_cur = sp.tile([B, 1], F32, tag="t")
                nc.vector.reciprocal(t_cur[:], cp[:])

        # last column update, transposed: c_row = s^T Ma  -> [1, B]
        crow_p = pr.tile([1, B], F32, tag="cp")
        nc.tensor.matmul(crow_p[:], s_cur[:], ma[:])
        trow = sg.tile([1, B], F32, tag="trow")
        nc.vector.reciprocal(trow[:], crow_p[:])

        # ---- P = diag(s) * (a M b) * diag(t), normalized -------------------
        tau = sg.tile([1, B], F32, tag="tau")  # b*t, row layout
        nc.vector.tensor_tensor(tau[:], trow[:], brow[:], mybir.AluOpType.mult)
        tb = pp.tile([L, B], F32, tag="pa")  # broadcast tau to 64 partitions
        nc.tensor.matmul(tb[:], ones_row, tau[:])
        p1 = sg.tile([L, B], F32, tag="p1")
        rowsum = sg.tile([L, 1], F32, tag="rowsum")
        nc.vector.scalar_tensor_tensor(
            p1[:], tb[:], s_cur[:], ma[:],
            mybir.AluOpType.mult, mybir.AluOpType.mult,
            accum_out=rowsum[:],
        )
        tot = pp.tile([1, 1], F32, tag="pb")
        nc.tensor.matmul(tot[:], rowsum[:], ones_col)
        invt = sg.tile([1, 1], F32, tag="invt")
        nc.vector.reciprocal(invt[:], tot[:])
        invb = pp.tile([L, 1], F32, tag="pb")  # broadcast 1/sum to 64 parts
        nc.tensor.matmul(invb[:], ones_row, invt[:])
        pout = sg.tile([L, B], F32, tag="pout")
        nc.vector.tensor_scalar(
            pout[:], p1[:], invb[:], None, mybir.AluOpType.mult
        )

        nc.sync.dma_start(out_d, pout[:])


_CACHE = {}


def _get_nc():
    if "nc" not in _CACHE:
        _CACHE["nc"] = _build()
    return _CACHE["nc"]


def _stage(inputs):
    trH = np.asarray(inputs["trH"], np.float32).reshape(L)
    wmax = np.asarray(inputs["wmax"], np.float32).reshape(L)
    a = np.asarray(inputs["a"], np.float32).reshape(L)
    theta = np.ascontiguousarray(np.asarray(inputs["theta"], np.float32))
    phi = np.asarray(inputs["phi"], np.float32).reshape(B)
    bits = np.asarray(inputs["bits"], np.int32).reshape(B)

    pk_col = np.zeros((L, _COL_LEN), np.float32)
    pk_col[:, 0] = a
    pk_col[:, 1] = 1.0
    pk_col[:B, 2] = phi
    pk_row = np.zeros((1, _ROW_LEN), np.float32)
    pk_row[0, _ROW_TRH : _ROW_TRH + L] = trH
    pk_row[0, _ROW_WMX : _ROW_WMX + L] = wmax
    pk_row[0, _ROW_ONE : _ROW_ONE + L] = 1.0
    pk_row[0, _ROW_PHI : _ROW_PHI + B] = phi
    return {
        "pk_col": pk_col,
        "pk_row": pk_row,
        "bits": bits,
        "theta": theta,
        "thetaT": np.ascontiguousarray(theta.T),
    }


def run(trace=False, **inputs):
    """Run on hardware; returns (output, BassKernelResults)."""
    nc = _get_nc()
    in_map = _stage(inputs)
    res = run_bass_kernel_spmd(
        nc,
        [dict(in_map) for _ in range(N_CORES)],
        core_ids=list(range(N_CORES)),
        trace=trace,
    )
    out = np.asarray(res.results[0]["out"], np.float32).reshape(L, B)
    return out, res


def kernel(**inputs) -> np.ndarray:
    out, _ = run(trace=False, **inputs)
    return out


# revision 11
# speedup vs baseline: 2.7058x; 2.7058x over previous
"""Trainium2 Bass kernel for nn_ChenDifferentiableAllocator (entropic OT / Sinkhorn).

Reference computes, from trH[64], wmax[64], a[64], theta[64,6], phi[6], bits[6]:
    C    = 0.5*trH[:,None] * ((2*wmax[:,None]/(2^bits-1))^2 / 12)
    K    = -(C - theta)/0.02 ; b = softmax(phi)
    200x log-domain Sinkhorn(K, log a, log b); P = exp(K+f+g); P /= P.sum()

The Sinkhorn fixed point is reached (to well below fp32 resolution) in ~16
alternating updates (measured: rel change < 1e-11 by iteration 25 in f64), so
the kernel runs the mathematically-identical multiplicative form to
convergence instead of 200 log-domain steps:

    M  = exp(K); Ma = a*M (lhsT for column update); MbT = (M*b)^T (lhsT for
    row update); iterate  s = 1/(Mb t), t = 1/(Ma^T s);  then
    P = diag(s) (a*M*b) diag(t) / sum.

Global normalization makes every positive rescaling of a and b cancel, so b
is used unnormalized (exp(phi), no softmax denominator needed).

Per-core work is a strictly serial PE<->DVE ping-pong (matvec / reciprocal).
The problem is replicated on all 8 cores; core 0's output is returned.

All arithmetic happens on-device. Host only restages layouts: packs the
small vectors into two staging tensors, transposes theta, supplies a ones
vector. 2^bits-1 is computed exactly on-device with integer ops
((bits+127)<<23 bit-cast to fp32).
"""

import numpy as np

import concourse.bass as bass
import concourse.tile as tile
from concourse import bacc, mybir
from concourse.bass_utils import run_bass_kernel_spmd

F32 = mybir.dt.float32
I32 = mybir.dt.int32

L, B = 64, 6
EPS = 0.02
N_ITERS = 17
N_CORES = 8
N_WARM_PRE = 5  # N=512 dummy matmuls before the preprocessing outer products
N_WARM_POST = 4  # and after, so PE busy-time crosses HAM's ~3.4us warm window

# pk_row layout (single partition strip): [trH(64) | wmax(64) | ones(64) | phi(6)]
_ROW_TRH = 0
_ROW_WMX = 64
_ROW_ONE = 128
_ROW_PHI = 192
_ROW_LEN = 198
# pk_col layout ([64, 3]): col0 = a, col1 = ones, col2 = phi padded to 64
_COL_LEN = 3


def _build():
    nc = bacc.Bacc("TRN2", target_bir_lowering=False, debug=False)

    pk_col_d = nc.dram_tensor("pk_col", [L, _COL_LEN], F32, kind="ExternalInput").ap()
    pk_row_d = nc.dram_tensor("pk_row", [1, _ROW_LEN], F32, kind="ExternalInput").ap()
    bits_d = nc.dram_tensor("bits", [B], I32, kind="ExternalInput").ap()
    theta_d = nc.dram_tensor("theta", [L, B], F32, kind="ExternalInput").ap()
    thetaT_d = nc.dram_tensor("thetaT", [B, L], F32, kind="ExternalInput").ap()
    out_d = nc.dram_tensor("out", [L, B], F32, kind="ExternalOutput").ap()

    with tile.TileContext(nc) as tc:
        _emit(tc, out_d, pk_col_d, pk_row_d, bits_d, theta_d, thetaT_d)

    nc.compile()
    return nc


def _emit(tc, out_d, pk_col_d, pk_row_d, bits_d, theta_d, thetaT_d):
    from contextlib import ExitStack

    nc = tc.nc
    ctx = ExitStack()
    with ctx:
        sg = ctx.enter_context(tc.tile_pool(name="sg", bufs=1))
        sp = ctx.enter_context(tc.tile_pool(name="sp", bufs=2))
        pp = ctx.enter_context(tc.tile_pool(name="pp", bufs=1, space="PSUM"))
        pr = ctx.enter_context(tc.tile_pool(name="pr", bufs=2, space="PSUM"))

        # ---- PE warm-up ----------------------------------------------------
        # The PE clock sits at 1.2 GHz (HAM-throttled) until ~3.4us of
        # sustained activity; the loop's duty cycle alone never trips it.
        # Run dep-free dummy matmuls during the input-DMA window so every
        # real matmul executes at 2.4 GHz.
        wsrc = sg.tile([1, 1], F32, tag="wsrc")
        nc.vector.memset(wsrc[:], 1.0)
        wps = pp.tile([1, 1], F32, tag="wm")
        for _ in range(N_WARM):
            nc.tensor.matmul(wps[:], wsrc[:], wsrc[:])

        # ---- input staging (spread across engine DMA queues) ---------------
        bits_r = sg.tile([1, B], I32, tag="bits_r")
        nc.sync.dma_start(bits_r[:], bits_d.unsqueeze(0))
        theta = sg.tile([L, B], F32, tag="theta")
        nc.sync.dma_start(theta[:], theta_d)
        pk_col = sg.tile([L, _COL_LEN], F32, tag="pk_col")
        nc.scalar.dma_start(pk_col[:], pk_col_d)
        pk_row = sg.tile([1, _ROW_LEN], F32, tag="pk_row")
        nc.scalar.dma_start(pk_row[:], pk_row_d)
        thetaT = sg.tile([B, L], F32, tag="thetaT")
        nc.gpsimd.dma_start(thetaT[:], thetaT_d)

        a_col = pk_col[:, 0:1]
        ones_col = pk_col[:, 1:2]
        phi_col = pk_col[0:B, 2:3]
        trh_row = pk_row[:, _ROW_TRH : _ROW_TRH + L]
        wmx_row = pk_row[:, _ROW_WMX : _ROW_WMX + L]
        ones_row = pk_row[:, _ROW_ONE : _ROW_ONE + L]
        phi_row = pk_row[:, _ROW_PHI : _ROW_PHI + B]

        # ---- preprocessing -------------------------------------------------
        # bcol/brow early: the first Exp triggers the ACT table load; let it
        # overlap the remaining input DMAs.
        bcol = sg.tile([B, 1], F32, tag="bcol")  # exp(phi), unnormalized b
        nc.scalar.activation(bcol[:], phi_col, mybir.ActivationFunctionType.Exp)
        brow = sg.tile([1, B], F32, tag="brow")
        nc.scalar.activation(brow[:], phi_row, mybir.ActivationFunctionType.Exp)

        # denom = 2^bits - 1, exactly: float_bits(2^b) = (b+127)*2^23, which is
        # exactly representable in fp32 (<= 8 mantissa bits), so compute it in
        # float, convert to int32, and bitcast.
        bits_f = sg.tile([1, B], F32, tag="bits_f")
        nc.vector.tensor_copy(bits_f[:], bits_r[:])
        ebits = sg.tile([1, B], F32, tag="ebits")
        nc.vector.tensor_scalar(
            ebits[:], bits_f[:], 127.0, float(1 << 23),
            mybir.AluOpType.add, mybir.AluOpType.mult,
        )
        p2i = sg.tile([1, B], I32, tag="p2i")
        nc.vector.tensor_copy(p2i[:], ebits[:])
        den = sg.tile([1, B], F32, tag="den")
        nc.vector.tensor_scalar(
            den[:], p2i[:].bitcast(F32), -1.0, None, mybir.AluOpType.add
        )
        densq = sg.tile([1, B], F32, tag="densq")
        nc.vector.tensor_tensor(densq[:], den[:], den[:], mybir.AluOpType.mult)
        colinv = sg.tile([1, B], F32, tag="colinv")  # 1/(2^b-1)^2
        nc.vector.reciprocal(colinv[:], densq[:])

        # rowe = trH*wmax^2/(6*EPS)  (row layout, for outer products)
        w2 = sg.tile([1, L], F32, tag="w2")
        nc.vector.tensor_tensor(w2[:], wmx_row, wmx_row, mybir.AluOpType.mult)
        rowe = sg.tile([1, L], F32, tag="rowe")
        nc.vector.scalar_tensor_tensor(
            rowe[:], w2[:], 1.0 / (6.0 * EPS), trh_row,
            mybir.AluOpType.mult, mybir.AluOpType.mult,
        )

        # Z1 = outer(rowe, colinv) [64,6];  Z2 = outer(colinv, rowe) [6,64]
        z1 = pp.tile([L, B], F32, tag="pa")
        nc.tensor.matmul(z1[:], rowe[:], colinv[:])
        z2 = pp.tile([B, L], F32, tag="pb")
        nc.tensor.matmul(z2[:], colinv[:], rowe[:])

        # K = theta/EPS - Z ;  M = exp(K) ;  MbT = exp(KT + phi_j) = (M*b)^T
        karg = sg.tile([L, B], F32, tag="karg")
        nc.vector.scalar_tensor_tensor(
            karg[:], theta[:], 1.0 / EPS, z1[:],
            mybir.AluOpType.mult, mybir.AluOpType.subtract,
        )
        kargT = sg.tile([B, L], F32, tag="kargT")
        nc.vector.scalar_tensor_tensor(
            kargT[:], thetaT[:], 1.0 / EPS, z2[:],
            mybir.AluOpType.mult, mybir.AluOpType.subtract,
        )
        m_mat = sg.tile([L, B], F32, tag="m_mat")
        nc.scalar.activation(m_mat[:], karg[:], mybir.ActivationFunctionType.Exp)
        mbT = sg.tile([B, L], F32, tag="mbT")  # lhsT of the row update
        nc.scalar.activation(
            mbT[:], kargT[:], mybir.ActivationFunctionType.Exp, bias=phi_col
        )
        ma = sg.tile([L, B], F32, tag="ma")  # lhsT of the column update
        nc.vector.tensor_scalar(
            ma[:], m_mat[:], a_col, None, mybir.AluOpType.mult
        )

        # ---- Sinkhorn loop -------------------------------------------------
        t_cur = sg.tile([B, 1], F32, tag="t0")
        nc.vector.reciprocal(t_cur[:], bcol[:])  # v0 = 1  =>  t0 = 1/b

        s_cur = None
        for i in range(N_ITERS):
            rp = pr.tile([L, 1], F32, tag="rp")
            nc.tensor.matmul(rp[:], mbT[:], t_cur[:])
            s_cur = sp.tile([L, 1], F32, tag="s")
            nc.vector.reciprocal(s_cur[:], rp[:])
            if i < N_ITERS - 1:
                cp = pr.tile([B, 1], F32, tag="cp")
                nc.tensor.matmul(cp[:], ma[:], s_cur[:])
                t_cur = sp.tile([B, 1], F32, tag="t")
                nc.vector.reciprocal(t_cur[:], cp[:])

        # last column update, transposed: c_row = s^T Ma  -> [1, B]
        crow_p = pr.tile([1, B], F32, tag="cp")
        nc.tensor.matmul(crow_p[:], s_cur[:], ma[:])
        trow = sg.tile([1, B], F32, tag="trow")
        nc.vector.reciprocal(trow[:], crow_p[:])

        # ---- P = diag(s) * (a M b) * diag(t), normalized -------------------
        tau = sg.tile([1, B], F32, tag="tau")  # b*t, row layout
        nc.vector.tensor_tensor(tau[:], trow[:], brow[:], mybir.AluOpType.mult)
        tb = pp.tile([L, B], F32, tag="pa")  # broadcast tau to 64 partitions
        nc.tensor.matmul(tb[:], ones_row, tau[:])
        p1 = sg.tile([L, B], F32, tag="p1")
        rowsum = sg.tile([L, 1], F32, tag="rowsum")
        nc.vector.scalar_tensor_tensor(
            p1[:], tb[:], s_cur[:], ma[:],
            mybir.AluOpType.mult, mybir.AluOpType.mult,
            accum_out=rowsum[:],
        )
        tot = pp.tile([1, 1], F32, tag="pb")
        nc.tensor.matmul(tot[:], rowsum[:], ones_col)
        invt = sg.tile([1, 1], F32, tag="invt")
        nc.vector.reciprocal(invt[:], tot[:])
        invb = pp.tile([L, 1], F32, tag="pb")  # broadcast 1/sum to 64 parts
        nc.tensor.matmul(invb[:], ones_row, invt[:])
        pout = sg.tile([L, B], F32, tag="pout")
        nc.vector.tensor_scalar(
            pout[:], p1[:], invb[:], None, mybir.AluOpType.mult
        )

        nc.sync.dma_start(out_d, pout[:])


_CACHE = {}


def _get_nc():
    if "nc" not in _CACHE:
        _CACHE["nc"] = _build()
    return _CACHE["nc"]


def _stage(inputs):
    trH = np.asarray(inputs["trH"], np.float32).reshape(L)
    wmax = np.asarray(inputs["wmax"], np.float32).reshape(L)
    a = np.asarray(inputs["a"], np.float32).reshape(L)
    theta = np.ascontiguousarray(np.asarray(inputs["theta"], np.float32))
    phi = np.asarray(inputs["phi"], np.float32).reshape(B)
    bits = np.asarray(inputs["bits"], np.int32).reshape(B)

    pk_col = np.zeros((L, _COL_LEN), np.float32)
    pk_col[:, 0] = a
    pk_col[:, 1] = 1.0
    pk_col[:B, 2] = phi
    pk_row = np.zeros((1, _ROW_LEN), np.float32)
    pk_row[0, _ROW_TRH : _ROW_TRH + L] = trH
    pk_row[0, _ROW_WMX : _ROW_WMX + L] = wmax
    pk_row[0, _ROW_ONE : _ROW_ONE + L] = 1.0
    pk_row[0, _ROW_PHI : _ROW_PHI + B] = phi
    return {
        "pk_col": pk_col,
        "pk_row": pk_row,
        "bits": bits,
        "theta": theta,
        "thetaT": np.ascontiguousarray(theta.T),
    }


def run(trace=False, **inputs):
    """Run on hardware; returns (output, BassKernelResults)."""
    nc = _get_nc()
    in_map = _stage(inputs)
    res = run_bass_kernel_spmd(
        nc,
        [dict(in_map) for _ in range(N_CORES)],
        core_ids=list(range(N_CORES)),
        trace=trace,
    )
    out = np.asarray(res.results[0]["out"], np.float32).reshape(L, B)
    return out, res


def kernel(**inputs) -> np.ndarray:
    out, _ = run(trace=False, **inputs)
    return out
